# revision 2
# baseline (speedup 1.0000x reference)
"""Soft decision-tree forward kernel for Trainium2 (8 NeuronCores, SPMD).

Per core (16384 rows), fp16 data path, f32 accumulation, 16-chunk
software pipeline (1024 rows per chunk):
  1. z06 = [x|1]^T-tile @ G06          (PE, 8 matmuls/chunk, K=33, N=128)
  2. act06 = sigmoid(z06)              (ACT, one op per chunk, fp16 out)
  3. tree DP levels 1..6, batch-major  (DVE, 12 ops/chunk over 8-tile groups)
  4. P7 -> DRAM -> xbar-transpose back (DMA) giving node-major P7T [128, 1024]
  5. z7T = G7 @ xq, sig7T = sigmoid(z7T - T7)   (PE K=32 + ACT bias)
  6. R7T = P7T * sig7T                 (DVE, node-major)
  7. pT = A^T@P7T + B^T@R7T            (PE, col-tiled 4x, PSUM accumulate)
  8. pT drains -> SBUF -> DRAM         (DVE cast + per-drain DMA)

Node layout: level d's column k <-> heap node bitrev_d(k) (concat child
placement). All reorderings are baked into G06 / G7 / A / B host-side.
P7 DRAM row order is q = (c, p, g): c = chunk, p = row-in-tile,
g = tile-in-chunk; batch b = c*1024 + g*128 + p. xQ input carries x
columns pre-permuted to q order so node-major matmul reads are contiguous.
Back-half stages (z7T/sig7/R7T/finals) run with a 2-chunk lag so the PE
never head-blocks on an in-flight transpose.
"""

import sys

sys.path.insert(0, "/opt/trn_rl_repo")

import numpy as np

import concourse.bacc as bacc
import concourse.bass as bass
import concourse.mybir as mybir
import concourse.tile as tile
from concourse import bass_utils

# ---- problem constants (hardcoded per contract) ----
BATCH = 131072
N_FEAT = 32
N_CLASSES = 10
N_CORES = 8
R = BATCH // N_CORES          # 16384 rows per core
TILE = 128
N_TILES = R // TILE           # 128
CHUNK_TILES = 8               # tiles per chunk
CHUNK = CHUNK_TILES * TILE    # 1024 rows per chunk
N_CHUNKS = N_TILES // CHUNK_TILES  # 16
KDIM = N_FEAT + 1             # 33
FIN_SUB = 512                 # final matmul sub-chunk (one PSUM bank)
LAG = 2                       # chunk lag between front half and back half

F32 = mybir.dt.float32
F16 = mybir.dt.float16
SIGMOID = mybir.ActivationFunctionType.Sigmoid

_COMPILED = None


def _bitrev(k, bits):
    r = 0
    for _ in range(bits):
        r = (r << 1) | (k & 1)
        k >>= 1
    return r


def _host_prep(thresholds, feats, leaf_class):
    """G06 [33,128], G7 [32,128], negT7 [128,1], A/B [128,10] (device layout)."""
    G06 = np.zeros((KDIM, 128), dtype=np.float32)
    f0, t0 = int(feats[0]), float(thresholds[0])
    G06[f0, 0] = -1.0
    G06[N_FEAT, 0] = +t0
    G06[f0, 1] = +1.0
    G06[N_FEAT, 1] = -t0
    for d in range(1, 7):
        n = 1 << d
        start = n - 1
        for k in range(n):
            j = _bitrev(k, d)
            G06[int(feats[start + j]), n + k] = 1.0
            G06[N_FEAT, n + k] = -float(thresholds[start + j])
    G7 = np.zeros((N_FEAT, 128), dtype=np.float32)
    negT7 = np.zeros((128, 1), dtype=np.float32)
    start7 = 127
    for k in range(128):
        j = _bitrev(k, 7)
        G7[int(feats[start7 + j]), k] = 1.0
        negT7[k, 0] = -float(thresholds[start7 + j])
    Lc = np.empty(128, dtype=np.int64)
    Rc = np.empty(128, dtype=np.int64)
    for k in range(128):
        j7 = _bitrev(k, 7)
        Lc[k] = leaf_class[2 * j7]
        Rc[k] = leaf_class[2 * j7 + 1]
    A = np.zeros((128, N_CLASSES), dtype=np.float32)
    Bm = np.zeros((128, N_CLASSES), dtype=np.float32)
    A[np.arange(128), Lc] = 1.0
    Bm[np.arange(128), Rc] += 1.0
    Bm[np.arange(128), Lc] -= 1.0
    return G06, G7, negT7, A, Bm


def _build_program():
    nc = bacc.Bacc("TRN2", target_bir_lowering=False, debug=False,
                   num_devices=N_CORES)

    xT_d = nc.dram_tensor("xT", [KDIM, R], F16, kind="ExternalInput")
    xq_d = nc.dram_tensor("xQ", [N_FEAT, R], F16, kind="ExternalInput")
    g06_d = nc.dram_tensor("G06", [KDIM, 128], F16, kind="ExternalInput")
    g7_d = nc.dram_tensor("G7", [N_FEAT, 128], F16, kind="ExternalInput")
    negT7_d = nc.dram_tensor("negT7", [128, 1], F32, kind="ExternalInput")
    a_d = nc.dram_tensor("A", [128, N_CLASSES], F16, kind="ExternalInput")
    b_d = nc.dram_tensor("B", [128, N_CLASSES], F16, kind="ExternalInput")
    pt_d = nc.dram_tensor("pT", [N_CLASSES, R], F16, kind="ExternalOutput")

    with tile.TileContext(nc) as tc:
        with (
            tc.tile_pool(name="const", bufs=1) as cpool,
            tc.tile_pool(name="dram", bufs=2, space="DRAM") as dpool,
            tc.tile_pool(name="xin", bufs=4) as xin_pool,
            tc.tile_pool(name="act06", bufs=2) as act_pool,
            tc.tile_pool(name="ptree", bufs=1) as tree_pool,
            tc.tile_pool(name="p7", bufs=2) as p7_pool,
            tc.tile_pool(name="p7t", bufs=3) as p7t_pool,
            tc.tile_pool(name="sig", bufs=2) as sig_pool,
            tc.tile_pool(name="nm", bufs=2) as nm_pool,
            tc.tile_pool(name="ptout", bufs=2) as pt_pool,
            tc.tile_pool(name="zpsum", bufs=2, space="PSUM") as zpsum,
            tc.tile_pool(name="z7psum", bufs=1, space="PSUM") as z7psum,
            tc.tile_pool(name="fpsum", bufs=2, space="PSUM") as fpsum,
        ):
            g06 = cpool.tile([KDIM, 128], F16, tag="G06")
            nc.gpsimd.dma_start(g06[:], g06_d.ap()[:, :])
            g7 = cpool.tile([N_FEAT, 128], F16, tag="G7")
            nc.gpsimd.dma_start(g7[:], g7_d.ap()[:, :])
            negT7 = cpool.tile([128, 1], F32, tag="negT7")
            nc.gpsimd.dma_start(negT7[:], negT7_d.ap()[:, :])
            a_s = cpool.tile([128, N_CLASSES], F16, tag="A")
            nc.gpsimd.dma_start(a_s[:], a_d.ap()[:, :])
            b_s = cpool.tile([128, N_CLASSES], F16, tag="B")
            nc.gpsimd.dma_start(b_s[:], b_d.ap()[:, :])

            xt_tiles = [None] * N_CHUNKS
            xq_tiles = [None] * N_CHUNKS

            def emit_load(c):
                base = c * CHUNK
                xt = xin_pool.tile([KDIM, CHUNK], F16, tag="xT", name="xt")
                nc.gpsimd.dma_start(xt[:], xT_d.ap()[:, base:base + CHUNK])
                xq = xin_pool.tile([N_FEAT, CHUNK], F16, tag="xQ", name="xq")
                nc.gpsimd.dma_start(xq[:], xq_d.ap()[:, base:base + CHUNK])
                xt_tiles[c] = xt
                xq_tiles[c] = xq

            fin_state = {}

            def emit_front(c):
                """z06 matmuls + sigmoid + DP + P7 write + transpose read."""
                act06 = act_pool.tile([TILE, CHUNK_TILES, 128], F16,
                                      tag="act06", name="act06")
                z06 = zpsum.tile([TILE, CHUNK_TILES * 128], F32, tag="z06",
                                 name="z06")
                for i in range(CHUNK_TILES):
                    nc.tensor.matmul(
                        z06[:, bass.ts(i, 128)],
                        xt_tiles[c][:, bass.ts(i, TILE)], g06[:],
                        start=True, stop=True,
                    )
                nc.scalar.activation(act06[:], z06[:], SIGMOID)
                # tree DP levels 1..6 (batch-major)
                prev = act06[:, :, 0:2]
                for d in range(1, 7):
                    n = 1 << d
                    if d < 6:
                        cur = tree_pool.tile([TILE, CHUNK_TILES, 2 * n], F16,
                                             tag=f"P{d + 1}", name=f"P{d + 1}")
                    else:
                        cur = p7_pool.tile([TILE, CHUNK_TILES, 128], F16,
                                           tag="P7", name="P7")
                    nc.vector.tensor_mul(
                        cur[:, :, n:2 * n], prev[:], act06[:, :, n:2 * n])
                    nc.vector.tensor_sub(
                        cur[:, :, 0:n], prev[:], cur[:, :, n:2 * n])
                    prev = cur[:, :, :]
                # P7 -> DRAM (contiguous per partition), then xbar transpose
                p7dram = dpool.tile([CHUNK, 128], F16, tag="p7dram",
                                    name="p7dram")
                p7dv = p7dram[:, :].rearrange("(p g) j -> p g j",
                                              p=TILE, g=CHUNK_TILES)
                nc.gpsimd.dma_start(p7dv, prev[:])
                p7t = p7t_pool.tile([128, CHUNK], F16, tag="P7T", name="p7t")
                nc.sync.dma_start_transpose(p7t[:], p7dram[:, :])
                return p7t

            def emit_back(c, p7t):
                """z7T + sig7 + R7T + finals (+ drain every 2nd chunk)."""
                z7t = z7psum.tile([128, CHUNK], F32, tag="z7T", name="z7t")
                for hf in range(CHUNK // FIN_SUB):
                    lo = c * CHUNK + hf * FIN_SUB
                    nc.tensor.matmul(z7t[:, bass.ts(hf, FIN_SUB)],
                                     g7[:], xq_tiles[c][:, bass.ts(hf, FIN_SUB)],
                                     start=True, stop=True)
                xq_tiles[c] = None
                sig7 = sig_pool.tile([128, CHUNK], F16, tag="sig7",
                                     name="sig7")
                nc.scalar.activation(sig7[:], z7t[:], SIGMOID, bias=negT7[:])
                r7t = nm_pool.tile([128, CHUNK], F16, tag="R7T", name="r7t")
                nc.vector.tensor_mul(r7t[:], p7t[:], sig7[:])
                for half in range(CHUNK // FIN_SUB):
                    u = c * (CHUNK // FIN_SUB) + half  # 0..31
                    k, cg = divmod(u, 4)
                    if cg == 0:
                        fin_state["fp"] = fpsum.tile(
                            [128, FIN_SUB], F32, tag="fin", name="fin")
                    fp = fin_state["fp"]
                    out_sl = fp[32 * cg:32 * cg + N_CLASSES, :]
                    nc.tensor.matmul(out_sl, a_s[:],
                                     p7t[:, bass.ts(half, FIN_SUB)],
                                     start=True, stop=False,
                                     tile_position=(0, 32 * cg))
                    nc.tensor.matmul(out_sl, b_s[:],
                                     r7t[:, bass.ts(half, FIN_SUB)],
                                     start=False, stop=True,
                                     tile_position=(0, 32 * cg))
                    if cg == 3:
                        pt_out = pt_pool.tile([128, FIN_SUB], F16,
                                              tag="ptout", name="ptout")
                        nc.vector.tensor_copy(pt_out[:], fp[:, 0:FIN_SUB])
                        # out: pT[cc, q], q = (4k+cg)*512+scol
                        pt_v = pt_d.ap()[:, k * 4 * FIN_SUB:
                                         (k + 1) * 4 * FIN_SUB].rearrange(
                            "cc (cg scol) -> cc cg scol", cg=4, scol=FIN_SUB)
                        for cg2 in range(4):
                            nc.gpsimd.dma_start(
                                pt_v[:, cg2, :],
                                pt_out[32 * cg2:32 * cg2 + N_CLASSES, :])

            # ---- pipelined main loop ----
            pending = []  # (chunk, p7t) waiting for back half
            for c in range(min(LAG + 1, N_CHUNKS)):
                emit_load(c)
            for c in range(N_CHUNKS):
                if c + LAG + 1 < N_CHUNKS:
                    emit_load(c + LAG + 1)
                pending.append((c, emit_front(c)))
                if c >= LAG:
                    bc, p7t = pending.pop(0)
                    emit_back(bc, p7t)
            while pending:
                bc, p7t = pending.pop(0)
                emit_back(bc, p7t)

    nc.compile()
    return nc


def _get_compiled():
    global _COMPILED
    if _COMPILED is None:
        _COMPILED = _build_program()
    return _COMPILED


def kernel(x, thresholds, feats, leaf_class, _trace=False):
    x = np.asarray(x, dtype=np.float32)
    thresholds = np.asarray(thresholds, dtype=np.float32)
    feats = np.asarray(feats, dtype=np.int32)
    leaf_class = np.asarray(leaf_class, dtype=np.int32)
    assert x.shape == (BATCH, N_FEAT)

    G06, G7, negT7, A, Bm = _host_prep(thresholds, feats, leaf_class)
    f16 = np.float16

    x_ext_T = np.empty((KDIM, BATCH), dtype=f16)
    x_ext_T[:N_FEAT, :] = x.T.astype(f16)
    x_ext_T[N_FEAT, :] = 1.0

    # per-chunk q-order: within chunk c (8 tiles), local
    # q = p*8 + g  <->  local b = g*128 + p
    b_of_q = np.empty(R, dtype=np.int64)
    ql = np.arange(CHUNK)
    loc = (ql % CHUNK_TILES) * TILE + ql // CHUNK_TILES
    for c in range(N_CHUNKS):
        b_of_q[c * CHUNK + ql] = c * CHUNK + loc

    in_maps = []
    for c in range(N_CORES):
        sl = slice(c * R, (c + 1) * R)
        xt_c = np.ascontiguousarray(x_ext_T[:, sl])
        xq_c = np.ascontiguousarray(xt_c[:N_FEAT][:, b_of_q])
        in_maps.append({
            "xT": xt_c,
            "xQ": xq_c,
            "G06": G06.astype(f16),
            "G7": G7.astype(f16),
            "negT7": negT7,
            "A": A.astype(f16),
            "B": Bm.astype(f16),
        })

    nc = _get_compiled()
    res = bass_utils.run_bass_kernel_spmd(
        nc, in_maps, core_ids=list(range(N_CORES)),
        trace=_trace, trace_cores=[0] if _trace else None,
    )

    out = np.empty((BATCH, N_CLASSES), dtype=np.float32)
    for c in range(N_CORES):
        sl = slice(c * R, (c + 1) * R)
        pt = res.results[c]["pT"].astype(np.float32)  # [10, R], q-order cols
        out[sl][b_of_q, :] = pt.T
    if _trace:
        kernel._last_results = res
    return out


# revision 5
# speedup vs baseline: 1.4341x; 1.4341x over previous
"""Soft decision-tree forward kernel for Trainium2 (8 NeuronCores, SPMD).

Per core (16384 rows), fp16 data path, f32 accumulation. 4 supertiles of
32 tiles (4096 rows); front half (z06/sigmoid/tree-DP) runs at supertile
granularity, the DRAM-transpose and back half (z7T/sig7/R7T/finals) at
quarter granularity (1024 rows), interleaved one supertile behind so the
PE stream never head-blocks on an in-flight transpose:
  1. z06 = [x|1]^T-tile @ G06          (PE, per 128-row tile, K=33, N=128)
  2. act06 = sigmoid(z06)              (ACT, one op per 8 tiles, fp16 out)
  3. tree DP levels 1..3 on GpSimd, 4..6 on DVE, batch-major
  4. P7 -> DRAM (one write/supertile) -> xbar-transpose back per quarter
  5. z7T = G7 @ xq-view, sig7T = sigmoid(z7T - T7)   (PE K=32 + ACT bias)
     (the q-order xq view is a rearranged AP on the resident xT tile --
      no second copy of x is shipped or loaded)
  6. R7T = P7T * sig7T                 (DVE, node-major)
  7. pT = A^T@P7T + B^T@R7T            (PE, col-tiled 4x, PSUM accumulate)
  8. pT -> SBUF (DVE cast) -> DRAM (4 strided DMAs at the end)

Node layout: level d's column k <-> heap node bitrev_d(k) (concat child
placement). All reorderings are baked into G06 / G7 / A / B host-side.
P7 DRAM row order is q = (s, p, g): s = supertile, p = row-in-tile,
g = tile-in-supertile; batch b = s*4096 + g*128 + p.
"""

import sys

sys.path.insert(0, "/opt/trn_rl_repo")

import numpy as np

import concourse.bacc as bacc
import concourse.bass as bass
import concourse.mybir as mybir
import concourse.tile as tile
from concourse import bass_utils

# ---- problem constants (hardcoded per contract) ----
BATCH = 131072
N_FEAT = 32
N_CLASSES = 10
N_CORES = 8
R = BATCH // N_CORES          # 16384 rows per core
TILE = 128
N_TILES = R // TILE           # 128
SUPER = 32                    # tiles per supertile
N_SUPER = N_TILES // SUPER    # 4
SROWS = SUPER * TILE          # 4096 rows per supertile
QROWS = 1024                  # back-half quarter (rows)
N_Q = SROWS // QROWS          # 4
KDIM = N_FEAT + 1             # 33
PSUM_TILES = 8                # z06 tiles per PSUM buffer -> ACT op of N=1024
FIN_SUB = 512                 # final matmul sub-chunk

F32 = mybir.dt.float32
F16 = mybir.dt.float16
SIGMOID = mybir.ActivationFunctionType.Sigmoid

_COMPILED = None


def _bitrev(k, bits):
    r = 0
    for _ in range(bits):
        r = (r << 1) | (k & 1)
        k >>= 1
    return r


def _host_prep(thresholds, feats, leaf_class):
    """G06 [33,128], G7 [32,128], negT7 [128,1], A/B [128,10] (device layout)."""
    G06 = np.zeros((KDIM, 128), dtype=np.float32)
    f0, t0 = int(feats[0]), float(thresholds[0])
    G06[f0, 0] = -1.0
    G06[N_FEAT, 0] = +t0
    G06[f0, 1] = +1.0
    G06[N_FEAT, 1] = -t0
    for d in range(1, 7):
        n = 1 << d
        start = n - 1
        for k in range(n):
            j = _bitrev(k, d)
            G06[int(feats[start + j]), n + k] = 1.0
            G06[N_FEAT, n + k] = -float(thresholds[start + j])
    G7 = np.zeros((N_FEAT, 128), dtype=np.float32)
    negT7 = np.zeros((128, 1), dtype=np.float32)
    start7 = 127
    for k in range(128):
        j = _bitrev(k, 7)
        G7[int(feats[start7 + j]), k] = 1.0
        negT7[k, 0] = -float(thresholds[start7 + j])
    Lc = np.empty(128, dtype=np.int64)
    Rc = np.empty(128, dtype=np.int64)
    for k in range(128):
        j7 = _bitrev(k, 7)
        Lc[k] = leaf_class[2 * j7]
        Rc[k] = leaf_class[2 * j7 + 1]
    A = np.zeros((128, N_CLASSES), dtype=np.float32)
    Bm = np.zeros((128, N_CLASSES), dtype=np.float32)
    A[np.arange(128), Lc] = 1.0
    Bm[np.arange(128), Rc] += 1.0
    Bm[np.arange(128), Lc] -= 1.0
    return G06, G7, negT7, A, Bm


def _build_program():
    nc = bacc.Bacc("TRN2", target_bir_lowering=False, debug=False,
                   num_devices=N_CORES)

    xT_d = nc.dram_tensor("xT", [KDIM, R], F16, kind="ExternalInput")
    g06_d = nc.dram_tensor("G06", [KDIM, 128], F16, kind="ExternalInput")
    g7_d = nc.dram_tensor("G7", [N_FEAT, 128], F16, kind="ExternalInput")
    negT7_d = nc.dram_tensor("negT7", [128, 1], F32, kind="ExternalInput")
    a_d = nc.dram_tensor("A", [128, N_CLASSES], F16, kind="ExternalInput")
    b_d = nc.dram_tensor("B", [128, N_CLASSES], F16, kind="ExternalInput")
    pt_d = nc.dram_tensor("pT", [N_CLASSES, R], F16, kind="ExternalOutput")

    with tile.TileContext(nc) as tc:
        with (
            tc.tile_pool(name="const", bufs=1) as cpool,
            tc.tile_pool(name="dram", bufs=2, space="DRAM") as dpool,
            tc.tile_pool(name="xin", bufs=3) as xin_pool,
            tc.tile_pool(name="act06", bufs=2) as act_pool,
            tc.tile_pool(name="ptree", bufs=1) as tree_pool,
            tc.tile_pool(name="p7", bufs=2) as p7_pool,
            tc.tile_pool(name="p7t", bufs=6) as p7t_pool,
            tc.tile_pool(name="sig", bufs=2) as sig_pool,
            tc.tile_pool(name="nm", bufs=2) as nm_pool,
            tc.tile_pool(name="ptout", bufs=1) as pt_pool,
            tc.tile_pool(name="zpsum", bufs=2, space="PSUM") as zpsum,
            tc.tile_pool(name="z7psum", bufs=1, space="PSUM") as z7psum,
            tc.tile_pool(name="fpsum", bufs=2, space="PSUM") as fpsum,
        ):
            g06 = cpool.tile([KDIM, 128], F16, tag="G06")
            nc.gpsimd.dma_start(g06[:], g06_d.ap()[:, :])
            g7 = cpool.tile([N_FEAT, 128], F16, tag="G7")
            nc.gpsimd.dma_start(g7[:], g7_d.ap()[:, :])
            negT7 = cpool.tile([128, 1], F32, tag="negT7")
            nc.gpsimd.dma_start(negT7[:], negT7_d.ap()[:, :])
            a_s = cpool.tile([128, N_CLASSES], F16, tag="A")
            nc.gpsimd.dma_start(a_s[:], a_d.ap()[:, :])
            b_s = cpool.tile([128, N_CLASSES], F16, tag="B")
            nc.gpsimd.dma_start(b_s[:], b_d.ap()[:, :])

            pt_out = pt_pool.tile([128, 8 * FIN_SUB], F16, tag="ptout")

            xt_tiles = [None] * N_SUPER

            def emit_load(s):
                xt = xin_pool.tile([KDIM, SROWS], F16, tag="xT", name="xt")
                nc.gpsimd.dma_start(
                    xt[:], xT_d.ap()[:, s * SROWS:(s + 1) * SROWS])
                xt_tiles[s] = xt

            fin_state = {}

            def emit_front_quarter(s, q, act06):
                """One 8-tile z06 PSUM group + its sigmoid."""
                z06 = zpsum.tile([TILE, PSUM_TILES * 128], F32, tag="z06",
                                 name="z06")
                for i in range(PSUM_TILES):
                    lt = q * PSUM_TILES + i
                    nc.tensor.matmul(
                        z06[:, bass.ts(i, 128)],
                        xt_tiles[s][:, bass.ts(lt, TILE)], g06[:],
                        start=True, stop=True,
                    )
                nc.scalar.activation(
                    act06[:, bass.ts(q, PSUM_TILES), :], z06[:], SIGMOID)

            def emit_dp_and_transpose(s, act06):
                """Tree DP levels 1..6 + P7 write + 4 transpose reads."""
                prev = act06[:, :, 0:2]
                for d in range(1, 7):
                    n = 1 << d
                    if d < 6:
                        cur = tree_pool.tile([TILE, SUPER, 2 * n], F16,
                                             tag=f"P{d + 1}", name=f"P{d + 1}")
                    else:
                        cur = p7_pool.tile([TILE, SUPER, 128], F16,
                                           tag="P7", name="P7")
                    eng = nc.gpsimd if d <= 3 else nc.vector
                    eng.tensor_mul(
                        cur[:, :, n:2 * n], prev[:], act06[:, :, n:2 * n])
                    eng.tensor_sub(
                        cur[:, :, 0:n], prev[:], cur[:, :, n:2 * n])
                    prev = cur[:, :, :]
                p7dram = dpool.tile([SROWS, 128], F16, tag="p7dram",
                                    name="p7dram")
                p7dv = p7dram[:, :].rearrange("(p g) j -> p g j",
                                              p=TILE, g=SUPER)
                nc.gpsimd.dma_start(p7dv, prev[:])
                p7ts = []
                for q in range(N_Q):
                    p7t = p7t_pool.tile([128, QROWS], F16, tag="P7T",
                                        name="p7t")
                    nc.sync.dma_start_transpose(
                        p7t[:], p7dram[q * QROWS:(q + 1) * QROWS, :])
                    p7ts.append(p7t)
                return p7ts

            def emit_back_quarter(s, q, p7t):
                """z7T + sig7 + R7T + finals for quarter q of supertile s."""
                # q-order xq view: column q-index = p*SUPER + g, so quarter q
                # covers p in [32q, 32q+32); matmul rhs reads xT via rearrange.
                xqv = xt_tiles[s][0:N_FEAT, :].rearrange(
                    "f (g p) -> f p g", g=SUPER, p=TILE)
                z7t = z7psum.tile([128, QROWS], F32, tag="z7T", name="z7t")
                for hf in range(QROWS // FIN_SUB):
                    p_lo = 32 * q + 16 * hf
                    nc.tensor.matmul(z7t[:, bass.ts(hf, FIN_SUB)],
                                     g7[:], xqv[:, p_lo:p_lo + 16, :],
                                     start=True, stop=True)
                sig7 = sig_pool.tile([128, QROWS], F16, tag="sig7",
                                     name="sig7")
                nc.scalar.activation(sig7[:], z7t[:], SIGMOID, bias=negT7[:])
                r7t = nm_pool.tile([128, QROWS], F16, tag="R7T", name="r7t")
                nc.vector.tensor_mul(r7t[:], p7t[:], sig7[:])
                for half in range(QROWS // FIN_SUB):
                    u = (s * N_Q + q) * (QROWS // FIN_SUB) + half  # 0..31
                    k, cg = divmod(u, 4)
                    if cg == 0:
                        fin_state["fp"] = fpsum.tile(
                            [128, FIN_SUB], F32, tag="fin", name="fin")
                    fp = fin_state["fp"]
                    out_sl = fp[32 * cg:32 * cg + N_CLASSES, :]
                    nc.tensor.matmul(out_sl, a_s[:],
                                     p7t[:, bass.ts(half, FIN_SUB)],
                                     start=True, stop=False,
                                     tile_position=(0, 32 * cg))
                    nc.tensor.matmul(out_sl, b_s[:],
                                     r7t[:, bass.ts(half, FIN_SUB)],
                                     start=False, stop=True,
                                     tile_position=(0, 32 * cg))
                    if cg == 3:
                        nc.vector.tensor_copy(
                            pt_out[:, bass.ts(k, FIN_SUB)], fp[:, 0:FIN_SUB])

            # ---- pipelined main loop (back half lags one supertile) ----
            emit_load(0)
            emit_load(1)
            pending = None  # (s, [p7t quarters])
            for s in range(N_SUPER):
                if s + 2 < N_SUPER:
                    emit_load(s + 2)
                act06 = act_pool.tile([TILE, SUPER, 128], F16, tag="act06",
                                      name="act06")
                for q in range(N_Q):
                    emit_front_quarter(s, q, act06)
                    if pending is not None:
                        emit_back_quarter(pending[0], q, pending[1][q])
                p7ts = emit_dp_and_transpose(s, act06)
                pending = (s, p7ts)
            for q in range(N_Q):
                emit_back_quarter(pending[0], q, pending[1][q])

            # ---- output DMA: 4 strided DMAs, one per col-group ----
            # pt_out[32*cg + c, k*512 + scol] = pT[c, q], q = (4k+cg)*512+scol
            pt_v = pt_d.ap().rearrange("c (k cg scol) -> c k cg scol",
                                       k=8, cg=4, scol=FIN_SUB)
            for cg in range(4):
                src_ap = pt_out[32 * cg:32 * cg + N_CLASSES, :].rearrange(
                    "c (k scol) -> c k scol", k=8, scol=FIN_SUB)
                nc.gpsimd.dma_start(pt_v[:, :, cg, :], src_ap)

    nc.compile()
    return nc


def _get_compiled():
    global _COMPILED
    if _COMPILED is None:
        _COMPILED = _build_program()
    return _COMPILED


def kernel(x, thresholds, feats, leaf_class, _trace=False):
    x = np.asarray(x, dtype=np.float32)
    thresholds = np.asarray(thresholds, dtype=np.float32)
    feats = np.asarray(feats, dtype=np.int32)
    leaf_class = np.asarray(leaf_class, dtype=np.int32)
    assert x.shape == (BATCH, N_FEAT)

    G06, G7, negT7, A, Bm = _host_prep(thresholds, feats, leaf_class)
    f16 = np.float16

    x_ext_T = np.empty((KDIM, BATCH), dtype=f16)
    x_ext_T[:N_FEAT, :] = x.T.astype(f16)
    x_ext_T[N_FEAT, :] = 1.0

    # per-supertile q-order: local q = p*SUPER + g  <->  local b = g*128 + p
    b_of_q = np.empty(R, dtype=np.int64)
    ql = np.arange(SROWS)
    loc = (ql % SUPER) * TILE + ql // SUPER
    for s in range(N_SUPER):
        b_of_q[s * SROWS + ql] = s * SROWS + loc

    in_maps = []
    for c in range(N_CORES):
        sl = slice(c * R, (c + 1) * R)
        xt_c = np.ascontiguousarray(x_ext_T[:, sl])
        in_maps.append({
            "xT": xt_c,
            "G06": G06.astype(f16),
            "G7": G7.astype(f16),
            "negT7": negT7,
            "A": A.astype(f16),
            "B": Bm.astype(f16),
        })

    nc = _get_compiled()
    res = bass_utils.run_bass_kernel_spmd(
        nc, in_maps, core_ids=list(range(N_CORES)),
        trace=_trace, trace_cores=[0] if _trace else None,
    )

    out = np.empty((BATCH, N_CLASSES), dtype=np.float32)
    for c in range(N_CORES):
        sl = slice(c * R, (c + 1) * R)
        pt = res.results[c]["pT"].astype(np.float32)  # [10, R], q-order cols
        out[sl][b_of_q, :] = pt.T
    if _trace:
        kernel._last_results = res
    return out


# revision 6
# speedup vs baseline: 1.6481x; 1.1492x over previous
"""Soft decision-tree forward kernel for Trainium2 (8 NeuronCores, SPMD).

Per core (16384 rows), fp16 data path, f32 accumulation. 4 supertiles of
32 tiles (4096 rows); front half (z06/sigmoid/tree-DP) runs at supertile
granularity, the DRAM-transpose and back half (z7T/sig7/R7T/finals) at
quarter granularity (1024 rows), interleaved one supertile behind so the
PE stream never head-blocks on an in-flight transpose:
  1. z06 = [x|1]^T-tile @ G06          (PE, per 128-row tile, K=33, N=128)
  2. act06 = sigmoid(z06)              (ACT, one op per 8 tiles, fp16 out)
  3. tree DP levels 1..3 on GpSimd, 4..6 on DVE, batch-major
  4. P7 -> DRAM (one write/supertile) -> xbar-transpose back per quarter
  5. z7T = G7 @ xq-view, sig7T = sigmoid(z7T - T7)   (PE K=32 + ACT bias)
     (the q-order xq view is a rearranged AP on the resident xT tile --
      no second copy of x is shipped or loaded)
  6. R7T = P7T * sig7T                 (DVE, node-major)
  7. pT = A^T@P7T + B^T@R7T            (PE, col-tiled 4x, PSUM accumulate)
  8. pT -> SBUF (DVE cast) -> DRAM (4 strided DMAs at the end)

Node layout: level d's column k <-> heap node bitrev_d(k) (concat child
placement). All reorderings are baked into G06 / G7 / A / B host-side.
P7 DRAM row order is q = (s, p, g): s = supertile, p = row-in-tile,
g = tile-in-supertile; batch b = s*4096 + g*128 + p.
"""

import sys

sys.path.insert(0, "/opt/trn_rl_repo")

import numpy as np

import concourse.bacc as bacc
import concourse.bass as bass
import concourse.mybir as mybir
import concourse.tile as tile
from concourse import bass_utils

# ---- problem constants (hardcoded per contract) ----
BATCH = 131072
N_FEAT = 32
N_CLASSES = 10
N_CORES = 8
R = BATCH // N_CORES          # 16384 rows per core
TILE = 128
N_TILES = R // TILE           # 128
SUPER = 32                    # tiles per supertile
N_SUPER = N_TILES // SUPER    # 4
SROWS = SUPER * TILE          # 4096 rows per supertile
QROWS = 1024                  # back-half quarter (rows)
N_Q = SROWS // QROWS          # 4
KDIM = N_FEAT + 1             # 33
PSUM_TILES = 8                # z06 tiles per PSUM buffer -> ACT op of N=1024
FIN_SUB = 512                 # final matmul sub-chunk

F32 = mybir.dt.float32
F16 = mybir.dt.float16
SIGMOID = mybir.ActivationFunctionType.Sigmoid

_COMPILED = None


def _bitrev(k, bits):
    r = 0
    for _ in range(bits):
        r = (r << 1) | (k & 1)
        k >>= 1
    return r


def _host_prep(thresholds, feats, leaf_class):
    """G06 [33,128], G7 [32,128], negT7 [128,1], A/B [128,10] (device layout)."""
    G06 = np.zeros((KDIM, 128), dtype=np.float32)
    f0, t0 = int(feats[0]), float(thresholds[0])
    G06[f0, 0] = -1.0
    G06[N_FEAT, 0] = +t0
    G06[f0, 1] = +1.0
    G06[N_FEAT, 1] = -t0
    for d in range(1, 7):
        n = 1 << d
        start = n - 1
        for k in range(n):
            j = _bitrev(k, d)
            G06[int(feats[start + j]), n + k] = 1.0
            G06[N_FEAT, n + k] = -float(thresholds[start + j])
    G7 = np.zeros((N_FEAT, 128), dtype=np.float32)
    negT7 = np.zeros((128, 1), dtype=np.float32)
    start7 = 127
    for k in range(128):
        j = _bitrev(k, 7)
        G7[int(feats[start7 + j]), k] = 1.0
        negT7[k, 0] = -float(thresholds[start7 + j])
    Lc = np.empty(128, dtype=np.int64)
    Rc = np.empty(128, dtype=np.int64)
    for k in range(128):
        j7 = _bitrev(k, 7)
        Lc[k] = leaf_class[2 * j7]
        Rc[k] = leaf_class[2 * j7 + 1]
    A = np.zeros((128, N_CLASSES), dtype=np.float32)
    Bm = np.zeros((128, N_CLASSES), dtype=np.float32)
    A[np.arange(128), Lc] = 1.0
    Bm[np.arange(128), Rc] += 1.0
    Bm[np.arange(128), Lc] -= 1.0
    return G06, G7, negT7, A, Bm


def _build_program():
    nc = bacc.Bacc("TRN2", target_bir_lowering=False, debug=False,
                   num_devices=N_CORES)

    xT_d = nc.dram_tensor("xT", [KDIM, R], F16, kind="ExternalInput")
    xq_d = nc.dram_tensor("xQ", [N_FEAT, R], F16, kind="ExternalInput")
    g06_d = nc.dram_tensor("G06", [KDIM, 128], F16, kind="ExternalInput")
    g7_d = nc.dram_tensor("G7", [N_FEAT, 128], F16, kind="ExternalInput")
    negT7_d = nc.dram_tensor("negT7", [128, 1], F32, kind="ExternalInput")
    a_d = nc.dram_tensor("A", [128, N_CLASSES], F16, kind="ExternalInput")
    b_d = nc.dram_tensor("B", [128, N_CLASSES], F16, kind="ExternalInput")
    pt_d = nc.dram_tensor("pT", [N_CLASSES, R], F16, kind="ExternalOutput")

    with tile.TileContext(nc) as tc:
        with (
            tc.tile_pool(name="const", bufs=1) as cpool,
            tc.tile_pool(name="dram", bufs=2, space="DRAM") as dpool,
            tc.tile_pool(name="xin", bufs=3) as xin_pool,
            tc.tile_pool(name="act06", bufs=3) as act_pool,
            tc.tile_pool(name="ptree", bufs=1) as tree_pool,
            tc.tile_pool(name="p7", bufs=2) as p7_pool,
            tc.tile_pool(name="p7t", bufs=8) as p7t_pool,
            tc.tile_pool(name="sig", bufs=3) as sig_pool,
            tc.tile_pool(name="nm", bufs=3) as nm_pool,
            tc.tile_pool(name="ptout", bufs=1) as pt_pool,
            tc.tile_pool(name="zpsum", bufs=2, space="PSUM") as zpsum,
            tc.tile_pool(name="z7psum", bufs=1, space="PSUM") as z7psum,
            tc.tile_pool(name="fpsum", bufs=2, space="PSUM") as fpsum,
        ):
            g06 = cpool.tile([KDIM, 128], F16, tag="G06")
            nc.gpsimd.dma_start(g06[:], g06_d.ap()[:, :])
            g7 = cpool.tile([N_FEAT, 128], F16, tag="G7")
            nc.gpsimd.dma_start(g7[:], g7_d.ap()[:, :])
            negT7 = cpool.tile([128, 1], F32, tag="negT7")
            nc.gpsimd.dma_start(negT7[:], negT7_d.ap()[:, :])
            a_s = cpool.tile([128, N_CLASSES], F16, tag="A")
            nc.gpsimd.dma_start(a_s[:], a_d.ap()[:, :])
            b_s = cpool.tile([128, N_CLASSES], F16, tag="B")
            nc.gpsimd.dma_start(b_s[:], b_d.ap()[:, :])

            pt_out = pt_pool.tile([128, 8 * FIN_SUB], F16, tag="ptout")

            xt_tiles = [None] * N_SUPER
            xq_tiles = [None] * N_SUPER

            def emit_load(s):
                xt = xin_pool.tile([KDIM, SROWS], F16, tag="xT", name="xt")
                nc.gpsimd.dma_start(
                    xt[:], xT_d.ap()[:, s * SROWS:(s + 1) * SROWS])
                xt_tiles[s] = xt
                xq = xin_pool.tile([N_FEAT, SROWS], F16, tag="xQ", name="xq")
                nc.gpsimd.dma_start(
                    xq[:], xq_d.ap()[:, s * SROWS:(s + 1) * SROWS])
                xq_tiles[s] = xq

            fin_state = {}

            def emit_front_quarter(s, q, act06):
                """One 8-tile z06 PSUM group + its sigmoid."""
                z06 = zpsum.tile([TILE, PSUM_TILES * 128], F32, tag="z06",
                                 name="z06")
                for i in range(PSUM_TILES):
                    lt = q * PSUM_TILES + i
                    nc.tensor.matmul(
                        z06[:, bass.ts(i, 128)],
                        xt_tiles[s][:, bass.ts(lt, TILE)], g06[:],
                        start=True, stop=True,
                    )
                nc.scalar.activation(
                    act06[:, bass.ts(q, PSUM_TILES), :], z06[:], SIGMOID)

            def emit_dp_and_transpose(s, act06):
                """Tree DP levels 1..6 + P7 write + 4 transpose reads."""
                prev = act06[:, :, 0:2]
                for d in range(1, 7):
                    n = 1 << d
                    if d < 6:
                        cur = tree_pool.tile([TILE, SUPER, 2 * n], F16,
                                             tag=f"P{d + 1}", name=f"P{d + 1}")
                    else:
                        cur = p7_pool.tile([TILE, SUPER, 128], F16,
                                           tag="P7", name="P7")
                    eng = nc.gpsimd if d <= 2 else nc.vector
                    eng.tensor_mul(
                        cur[:, :, n:2 * n], prev[:], act06[:, :, n:2 * n])
                    eng.tensor_sub(
                        cur[:, :, 0:n], prev[:], cur[:, :, n:2 * n])
                    prev = cur[:, :, :]
                p7dram = dpool.tile([SROWS, 128], F16, tag="p7dram",
                                    name="p7dram")
                p7dv = p7dram[:, :].rearrange("(p g) j -> p g j",
                                              p=TILE, g=SUPER)
                nc.sync.dma_start(p7dv, prev[:])
                p7ts = []
                for q in range(N_Q):
                    p7t = p7t_pool.tile([128, QROWS], F16, tag="P7T",
                                        name="p7t")
                    nc.sync.dma_start_transpose(
                        p7t[:], p7dram[q * QROWS:(q + 1) * QROWS, :])
                    p7ts.append(p7t)
                return p7ts

            def emit_back_quarter(s, q, p7t):
                """z7T + sig7 + R7T + finals for quarter q of supertile s."""
                z7t = z7psum.tile([128, QROWS], F32, tag="z7T", name="z7t")
                for hf in range(QROWS // FIN_SUB):
                    lo = q * QROWS + hf * FIN_SUB
                    nc.tensor.matmul(z7t[:, bass.ts(hf, FIN_SUB)],
                                     g7[:], xq_tiles[s][:, lo:lo + FIN_SUB],
                                     start=True, stop=True)
                sig7 = sig_pool.tile([128, QROWS], F16, tag="sig7",
                                     name="sig7")
                nc.scalar.activation(sig7[:], z7t[:], SIGMOID, bias=negT7[:])
                r7t = nm_pool.tile([128, QROWS], F16, tag="R7T", name="r7t")
                nc.vector.tensor_mul(r7t[:], p7t[:], sig7[:])
                for half in range(QROWS // FIN_SUB):
                    u = (s * N_Q + q) * (QROWS // FIN_SUB) + half  # 0..31
                    k, cg = divmod(u, 4)
                    if cg == 0:
                        fin_state["fp"] = fpsum.tile(
                            [128, FIN_SUB], F32, tag="fin", name="fin")
                    fp = fin_state["fp"]
                    out_sl = fp[32 * cg:32 * cg + N_CLASSES, :]
                    nc.tensor.matmul(out_sl, a_s[:],
                                     p7t[:, bass.ts(half, FIN_SUB)],
                                     start=True, stop=False,
                                     tile_position=(0, 32 * cg))
                    nc.tensor.matmul(out_sl, b_s[:],
                                     r7t[:, bass.ts(half, FIN_SUB)],
                                     start=False, stop=True,
                                     tile_position=(0, 32 * cg))
                    if cg == 3:
                        nc.vector.tensor_copy(
                            pt_out[:, bass.ts(k, FIN_SUB)], fp[:, 0:FIN_SUB])

            # ---- pipelined main loop (back half lags one supertile) ----
            emit_load(0)
            emit_load(1)
            pending = None  # (s, [p7t quarters])
            for s in range(N_SUPER):
                if s + 2 < N_SUPER:
                    emit_load(s + 2)
                act06 = act_pool.tile([TILE, SUPER, 128], F16, tag="act06",
                                      name="act06")
                for q in range(N_Q):
                    emit_front_quarter(s, q, act06)
                    if pending is not None:
                        emit_back_quarter(pending[0], q, pending[1][q])
                p7ts = emit_dp_and_transpose(s, act06)
                pending = (s, p7ts)
            for q in range(N_Q):
                emit_back_quarter(pending[0], q, pending[1][q])

            # ---- output DMA: 4 strided DMAs, one per col-group ----
            # pt_out[32*cg + c, k*512 + scol] = pT[c, q], q = (4k+cg)*512+scol
            pt_v = pt_d.ap().rearrange("c (k cg scol) -> c k cg scol",
                                       k=8, cg=4, scol=FIN_SUB)
            for cg in range(4):
                src_ap = pt_out[32 * cg:32 * cg + N_CLASSES, :].rearrange(
                    "c (k scol) -> c k scol", k=8, scol=FIN_SUB)
                nc.gpsimd.dma_start(pt_v[:, :, cg, :], src_ap)

    nc.compile()
    return nc


def _get_compiled():
    global _COMPILED
    if _COMPILED is None:
        _COMPILED = _build_program()
    return _COMPILED


def kernel(x, thresholds, feats, leaf_class, _trace=False):
    x = np.asarray(x, dtype=np.float32)
    thresholds = np.asarray(thresholds, dtype=np.float32)
    feats = np.asarray(feats, dtype=np.int32)
    leaf_class = np.asarray(leaf_class, dtype=np.int32)
    assert x.shape == (BATCH, N_FEAT)

    G06, G7, negT7, A, Bm = _host_prep(thresholds, feats, leaf_class)
    f16 = np.float16

    x_ext_T = np.empty((KDIM, BATCH), dtype=f16)
    x_ext_T[:N_FEAT, :] = x.T.astype(f16)
    x_ext_T[N_FEAT, :] = 1.0

    # per-supertile q-order: local q = p*SUPER + g  <->  local b = g*128 + p
    b_of_q = np.empty(R, dtype=np.int64)
    ql = np.arange(SROWS)
    loc = (ql % SUPER) * TILE + ql // SUPER
    for s in range(N_SUPER):
        b_of_q[s * SROWS + ql] = s * SROWS + loc

    in_maps = []
    for c in range(N_CORES):
        sl = slice(c * R, (c + 1) * R)
        xt_c = np.ascontiguousarray(x_ext_T[:, sl])
        xq_c = np.ascontiguousarray(xt_c[:N_FEAT][:, b_of_q])
        in_maps.append({
            "xT": xt_c,
            "xQ": xq_c,
            "G06": G06.astype(f16),
            "G7": G7.astype(f16),
            "negT7": negT7,
            "A": A.astype(f16),
            "B": Bm.astype(f16),
        })

    nc = _get_compiled()
    res = bass_utils.run_bass_kernel_spmd(
        nc, in_maps, core_ids=list(range(N_CORES)),
        trace=_trace, trace_cores=[0] if _trace else None,
    )

    out = np.empty((BATCH, N_CLASSES), dtype=np.float32)
    for c in range(N_CORES):
        sl = slice(c * R, (c + 1) * R)
        pt = res.results[c]["pT"].astype(np.float32)  # [10, R], q-order cols
        out[sl][b_of_q, :] = pt.T
    if _trace:
        kernel._last_results = res
    return out


# revision 8
# speedup vs baseline: 1.7160x; 1.0412x over previous
"""Soft decision-tree forward kernel for Trainium2 (8 NeuronCores, SPMD).

Per core (16384 rows), fp16 data path, f32 accumulation. 4 supertiles of
32 tiles (4096 rows); front half (z06/sigmoid/tree-DP) runs at supertile
granularity, the DRAM-transpose and back half (z7T/sig7/R7T/finals) at
quarter granularity (1024 rows), interleaved one supertile behind so the
PE stream never head-blocks on an in-flight transpose:
  1. z06 = [x|1]^T-tile @ G06          (PE, per 128-row tile, K=33, N=128)
  2. act06 = sigmoid(z06)              (ACT, one op per 8 tiles, fp16 out)
  3. tree DP levels 1..3 on GpSimd, 4..6 on DVE, batch-major
  4. P7 -> DRAM (one write/supertile) -> xbar-transpose back per quarter
  5. z7T = G7 @ xq-view, sig7T = sigmoid(z7T - T7)   (PE K=32 + ACT bias)
     (the q-order xq view is a rearranged AP on the resident xT tile --
      no second copy of x is shipped or loaded)
  6. R7T = P7T * sig7T                 (DVE, node-major)
  7. pT = A^T@P7T + B^T@R7T            (PE, col-tiled 4x, PSUM accumulate)
  8. pT -> SBUF (DVE cast) -> DRAM (4 strided DMAs at the end)

Node layout: level d's column k <-> heap node bitrev_d(k) (concat child
placement). All reorderings are baked into G06 / G7 / A / B host-side.
P7 DRAM row order is q = (s, p, g): s = supertile, p = row-in-tile,
g = tile-in-supertile; batch b = s*4096 + g*128 + p.
"""

import sys

sys.path.insert(0, "/opt/trn_rl_repo")

import numpy as np

import concourse.bacc as bacc
import concourse.bass as bass
import concourse.mybir as mybir
import concourse.tile as tile
from concourse import bass_utils

# ---- problem constants (hardcoded per contract) ----
BATCH = 131072
N_FEAT = 32
N_CLASSES = 10
N_CORES = 8
R = BATCH // N_CORES          # 16384 rows per core
TILE = 128
N_TILES = R // TILE           # 128
SUPER = 32                    # tiles per supertile
N_SUPER = N_TILES // SUPER    # 4
SROWS = SUPER * TILE          # 4096 rows per supertile
QROWS = 1024                  # back-half quarter (rows)
N_Q = SROWS // QROWS          # 4
KDIM = N_FEAT + 1             # 33
PSUM_TILES = 8                # z06 tiles per PSUM buffer -> ACT op of N=1024
FIN_SUB = 512                 # final matmul sub-chunk

F32 = mybir.dt.float32
F16 = mybir.dt.float16
SIGMOID = mybir.ActivationFunctionType.Sigmoid

_COMPILED = None


def _bitrev(k, bits):
    r = 0
    for _ in range(bits):
        r = (r << 1) | (k & 1)
        k >>= 1
    return r


def _host_prep(thresholds, feats, leaf_class):
    """G06 [33,128], G7 [32,128], negT7 [128,1], A/B [128,10] (device layout)."""
    G06 = np.zeros((KDIM, 128), dtype=np.float32)
    f0, t0 = int(feats[0]), float(thresholds[0])
    G06[f0, 0] = -1.0
    G06[N_FEAT, 0] = +t0
    G06[f0, 1] = +1.0
    G06[N_FEAT, 1] = -t0
    for d in range(1, 7):
        n = 1 << d
        start = n - 1
        for k in range(n):
            j = _bitrev(k, d)
            G06[int(feats[start + j]), n + k] = 1.0
            G06[N_FEAT, n + k] = -float(thresholds[start + j])
    G7 = np.zeros((N_FEAT, 128), dtype=np.float32)
    negT7 = np.zeros((128, 1), dtype=np.float32)
    start7 = 127
    for k in range(128):
        j = _bitrev(k, 7)
        G7[int(feats[start7 + j]), k] = 1.0
        negT7[k, 0] = -float(thresholds[start7 + j])
    Lc = np.empty(128, dtype=np.int64)
    Rc = np.empty(128, dtype=np.int64)
    for k in range(128):
        j7 = _bitrev(k, 7)
        Lc[k] = leaf_class[2 * j7]
        Rc[k] = leaf_class[2 * j7 + 1]
    A = np.zeros((128, N_CLASSES), dtype=np.float32)
    Bm = np.zeros((128, N_CLASSES), dtype=np.float32)
    A[np.arange(128), Lc] = 1.0
    Bm[np.arange(128), Rc] += 1.0
    Bm[np.arange(128), Lc] -= 1.0
    return G06, G7, negT7, A, Bm


def _build_program():
    nc = bacc.Bacc("TRN2", target_bir_lowering=False, debug=False,
                   num_devices=N_CORES)

    xT_d = nc.dram_tensor("xT", [KDIM, R], F16, kind="ExternalInput")
    xq_d = nc.dram_tensor("xQ", [N_FEAT, R], F16, kind="ExternalInput")
    g06_d = nc.dram_tensor("G06", [KDIM, 128], F16, kind="ExternalInput")
    g7_d = nc.dram_tensor("G7", [N_FEAT, 128], F16, kind="ExternalInput")
    negT7_d = nc.dram_tensor("negT7", [128, 1], F32, kind="ExternalInput")
    a_d = nc.dram_tensor("A", [128, N_CLASSES], F16, kind="ExternalInput")
    b_d = nc.dram_tensor("B", [128, N_CLASSES], F16, kind="ExternalInput")
    pt_d = nc.dram_tensor("pT", [N_CLASSES, R], F16, kind="ExternalOutput")

    with tile.TileContext(nc) as tc:
        with (
            tc.tile_pool(name="const", bufs=1) as cpool,
            tc.tile_pool(name="dram", bufs=2, space="DRAM") as dpool,
            tc.tile_pool(name="xin", bufs=3) as xin_pool,
            tc.tile_pool(name="act06", bufs=3) as act_pool,
            tc.tile_pool(name="ptree", bufs=1) as tree_pool,
            tc.tile_pool(name="p7", bufs=2) as p7_pool,
            tc.tile_pool(name="p7t", bufs=8) as p7t_pool,
            tc.tile_pool(name="sig", bufs=3) as sig_pool,
            tc.tile_pool(name="nm", bufs=4) as nm_pool,
            tc.tile_pool(name="ptout", bufs=1) as pt_pool,
            tc.tile_pool(name="zpsum", bufs=2, space="PSUM") as zpsum,
            tc.tile_pool(name="z7psum", bufs=1, space="PSUM") as z7psum,
            tc.tile_pool(name="fpsum", bufs=2, space="PSUM") as fpsum,
        ):
            g06 = cpool.tile([KDIM, 128], F16, tag="G06")
            nc.gpsimd.dma_start(g06[:], g06_d.ap()[:, :])
            g7 = cpool.tile([N_FEAT, 128], F16, tag="G7")
            nc.gpsimd.dma_start(g7[:], g7_d.ap()[:, :])
            negT7 = cpool.tile([128, 1], F32, tag="negT7")
            nc.gpsimd.dma_start(negT7[:], negT7_d.ap()[:, :])
            a_s = cpool.tile([128, N_CLASSES], F16, tag="A")
            nc.gpsimd.dma_start(a_s[:], a_d.ap()[:, :])
            b_s = cpool.tile([128, N_CLASSES], F16, tag="B")
            nc.gpsimd.dma_start(b_s[:], b_d.ap()[:, :])

            pt_out = pt_pool.tile([128, 8 * FIN_SUB], F16, tag="ptout")

            xt_tiles = [None] * N_SUPER
            xq_tiles = [None] * N_SUPER

            def emit_load(s):
                xt = xin_pool.tile([KDIM, SROWS], F16, tag="xT", name="xt")
                nc.gpsimd.dma_start(
                    xt[:], xT_d.ap()[:, s * SROWS:(s + 1) * SROWS])
                xt_tiles[s] = xt
                xq = xin_pool.tile([N_FEAT, SROWS], F16, tag="xQ", name="xq")
                nc.gpsimd.dma_start(
                    xq[:], xq_d.ap()[:, s * SROWS:(s + 1) * SROWS])
                xq_tiles[s] = xq

            def emit_front_quarter(s, q, act06):
                """One 8-tile z06 PSUM group + its sigmoid. Each 128-row tile
                is split into two 64-batch halves col-tiled at (0,0)/(0,64)
                so consecutive matmuls execute concurrently in the array."""
                z06 = zpsum.tile([TILE, PSUM_TILES * 128], F32, tag="z06",
                                 name="z06")
                for i in range(PSUM_TILES):
                    lt = q * PSUM_TILES + i
                    for h in range(2):
                        nc.tensor.matmul(
                            z06[64 * h:64 * h + 64, bass.ts(i, 128)],
                            xt_tiles[s][:, lt * TILE + 64 * h:
                                        lt * TILE + 64 * h + 64],
                            g06[:],
                            start=True, stop=True,
                            tile_position=(0, 64 * h),
                        )
                nc.scalar.activation(
                    act06[:, bass.ts(q, PSUM_TILES), :], z06[:], SIGMOID)

            def emit_dp_and_transpose(s, act06):
                """Tree DP levels 1..6 + P7 write + 4 transpose reads."""
                prev = act06[:, :, 0:2]
                for d in range(1, 7):
                    n = 1 << d
                    if d < 6:
                        cur = tree_pool.tile([TILE, SUPER, 2 * n], F16,
                                             tag=f"P{d + 1}", name=f"P{d + 1}")
                    else:
                        cur = p7_pool.tile([TILE, SUPER, 128], F16,
                                           tag="P7", name="P7")
                    eng = nc.gpsimd if d <= 2 else nc.vector
                    eng.tensor_mul(
                        cur[:, :, n:2 * n], prev[:], act06[:, :, n:2 * n])
                    eng.tensor_sub(
                        cur[:, :, 0:n], prev[:], cur[:, :, n:2 * n])
                    prev = cur[:, :, :]
                p7dram = dpool.tile([SROWS, 128], F16, tag="p7dram",
                                    name="p7dram")
                p7dv = p7dram[:, :].rearrange("(p g) j -> p g j",
                                              p=TILE, g=SUPER)
                nc.sync.dma_start(p7dv, prev[:])
                p7ts = []
                for q in range(N_Q):
                    p7t = p7t_pool.tile([128, QROWS], F16, tag="P7T",
                                        name="p7t")
                    nc.sync.dma_start_transpose(
                        p7t[:], p7dram[q * QROWS:(q + 1) * QROWS, :])
                    p7ts.append(p7t)
                return p7ts

            def emit_back_quarter(s, q, p7t):
                """z7T + sig7 + R7T for quarter q of supertile s. The node
                dim is split at (0,0)/(0,64) for 2x array concurrency.
                Finals are deferred (emit_finals_group) one slot later."""
                z7t = z7psum.tile([128, QROWS], F32, tag="z7T", name="z7t")
                for hf in range(QROWS // FIN_SUB):
                    lo = q * QROWS + hf * FIN_SUB
                    for h in range(2):
                        nc.tensor.matmul(
                            z7t[64 * h:64 * h + 64, bass.ts(hf, FIN_SUB)],
                            g7[:, 64 * h:64 * h + 64],
                            xq_tiles[s][:, lo:lo + FIN_SUB],
                            start=True, stop=True,
                            tile_position=(0, 64 * h),
                        )
                sig7 = sig_pool.tile([128, QROWS], F16, tag="sig7",
                                     name="sig7")
                nc.scalar.activation(sig7[:], z7t[:], SIGMOID, bias=negT7[:])
                r7t = nm_pool.tile([128, QROWS], F16, tag="R7T", name="r7t")
                nc.vector.tensor_mul(r7t[:], p7t[:], sig7[:])
                return r7t

            def emit_finals_group(g, quarters):
                """Finals for 512-col subs u=4g..4g+3 (two 1024 quarters).
                A-matmuls for all 4 col-groups issue back-to-back (distinct
                col_grps -> concurrent execution), then the 4 B-matmuls."""
                fp = fpsum.tile([128, FIN_SUB], F32, tag="fin", name="fin")
                subs = []
                for du in range(4):
                    u = 4 * g + du
                    p7t, r7t = quarters[du // 2]
                    half = du % 2
                    subs.append((u % 4, p7t, r7t, half))
                for cg, p7t, r7t, half in subs:
                    nc.tensor.matmul(fp[32 * cg:32 * cg + N_CLASSES, :],
                                     a_s[:], p7t[:, bass.ts(half, FIN_SUB)],
                                     start=True, stop=False,
                                     tile_position=(0, 32 * cg))
                for cg, p7t, r7t, half in subs:
                    nc.tensor.matmul(fp[32 * cg:32 * cg + N_CLASSES, :],
                                     b_s[:], r7t[:, bass.ts(half, FIN_SUB)],
                                     start=False, stop=True,
                                     tile_position=(0, 32 * cg))
                nc.vector.tensor_copy(
                    pt_out[:, bass.ts(g, FIN_SUB)], fp[:, 0:FIN_SUB])

            # ---- pipelined main loop ----
            # back half (z7/sig7/R7T) lags the front by one supertile;
            # finals lag the back half by one further quarter-slot, grouped
            # over pairs of quarters (one PSUM col-group burst per pair).
            emit_load(0)
            emit_load(1)
            pending = None     # (s, [p7t quarters])
            done_q = []        # [(p7t, r7t)] per completed back quarter
            n_fin = 0          # finals groups emitted so far
            for s in range(N_SUPER):
                if s + 2 < N_SUPER:
                    emit_load(s + 2)
                act06 = act_pool.tile([TILE, SUPER, 128], F16, tag="act06",
                                      name="act06")
                for q in range(N_Q):
                    emit_front_quarter(s, q, act06)
                    if pending is not None:
                        r7t = emit_back_quarter(pending[0], q, pending[1][q])
                        done_q.append((pending[1][q], r7t))
                    if len(done_q) >= 2 * (n_fin + 1) + 1:
                        emit_finals_group(n_fin, done_q[2 * n_fin:
                                                        2 * n_fin + 2])
                        n_fin += 1
                p7ts = emit_dp_and_transpose(s, act06)
                pending = (s, p7ts)
            for q in range(N_Q):
                r7t = emit_back_quarter(pending[0], q, pending[1][q])
                done_q.append((pending[1][q], r7t))
                if len(done_q) >= 2 * (n_fin + 1) + 1:
                    emit_finals_group(n_fin, done_q[2 * n_fin:2 * n_fin + 2])
                    n_fin += 1
            while n_fin < 8:
                emit_finals_group(n_fin, done_q[2 * n_fin:2 * n_fin + 2])
                n_fin += 1

            # ---- output DMA: 4 strided DMAs, one per col-group ----
            # pt_out[32*cg + c, k*512 + scol] = pT[c, q], q = (4k+cg)*512+scol
            pt_v = pt_d.ap().rearrange("c (k cg scol) -> c k cg scol",
                                       k=8, cg=4, scol=FIN_SUB)
            for cg in range(4):
                src_ap = pt_out[32 * cg:32 * cg + N_CLASSES, :].rearrange(
                    "c (k scol) -> c k scol", k=8, scol=FIN_SUB)
                nc.gpsimd.dma_start(pt_v[:, :, cg, :], src_ap)

    nc.compile()
    return nc


def _get_compiled():
    global _COMPILED
    if _COMPILED is None:
        _COMPILED = _build_program()
    return _COMPILED


def kernel(x, thresholds, feats, leaf_class, _trace=False):
    x = np.asarray(x, dtype=np.float32)
    thresholds = np.asarray(thresholds, dtype=np.float32)
    feats = np.asarray(feats, dtype=np.int32)
    leaf_class = np.asarray(leaf_class, dtype=np.int32)
    assert x.shape == (BATCH, N_FEAT)

    G06, G7, negT7, A, Bm = _host_prep(thresholds, feats, leaf_class)
    f16 = np.float16

    x_ext_T = np.empty((KDIM, BATCH), dtype=f16)
    x_ext_T[:N_FEAT, :] = x.T.astype(f16)
    x_ext_T[N_FEAT, :] = 1.0

    # per-supertile q-order: local q = p*SUPER + g  <->  local b = g*128 + p
    b_of_q = np.empty(R, dtype=np.int64)
    ql = np.arange(SROWS)
    loc = (ql % SUPER) * TILE + ql // SUPER
    for s in range(N_SUPER):
        b_of_q[s * SROWS + ql] = s * SROWS + loc

    in_maps = []
    for c in range(N_CORES):
        sl = slice(c * R, (c + 1) * R)
        xt_c = np.ascontiguousarray(x_ext_T[:, sl])
        xq_c = np.ascontiguousarray(xt_c[:N_FEAT][:, b_of_q])
        in_maps.append({
            "xT": xt_c,
            "xQ": xq_c,
            "G06": G06.astype(f16),
            "G7": G7.astype(f16),
            "negT7": negT7,
            "A": A.astype(f16),
            "B": Bm.astype(f16),
        })

    nc = _get_compiled()
    res = bass_utils.run_bass_kernel_spmd(
        nc, in_maps, core_ids=list(range(N_CORES)),
        trace=_trace, trace_cores=[0] if _trace else None,
    )

    out = np.empty((BATCH, N_CLASSES), dtype=np.float32)
    for c in range(N_CORES):
        sl = slice(c * R, (c + 1) * R)
        pt = res.results[c]["pT"].astype(np.float32)  # [10, R], q-order cols
        out[sl][b_of_q, :] = pt.T
    if _trace:
        kernel._last_results = res
    return out
